# revision 10
# baseline (speedup 1.0000x reference)
"""Trainium2 Bass kernel for nn_CSCLoss: multi-scale bilinear point-sampling
cosine-consistency loss.

loss = 1 - mean_{pairs,(b,n)} <normalize(sample(feat_i, p_bn)), normalize(sample(feat_j, p_bn))>

Sharding: data-parallel over batch - 32 images -> 8 cores x 4 images; the
host sums the 8 per-core partial sums and applies the loss epilogue.

Per-core dataflow. ap_gather cost is ~27ns per INDEX nearly independent of
d (measured), so the design minimizes index count:
 - l2/l1 stream into tiles holding [A_s0|B_s0|A_s1|B_s1] where B = A shifted
   by one element (built by SBUF->SBUF DMA, no extra HBM). Any x-pair
   (p, p+1) is then an even-aligned d=2 block: of A if p is even, of B if p
   is odd. One gather index per (point, row, channel-chunk): 512 idx/level
   instead of 1024.
 - l0 (16.8 MB/core) keeps d=1 4-corner gathers (a B copy would cost too
   much SBUF/DMA): 8 per-(image,chunk) tiles of [128,4096], 1024 idx total.
 - All 12 stream DMAs ride the scalar HWDGE queue in arrival order; the
   sync queue carries only boxes, the 4 B-copies, index/weight staging and
   the result, so small transfers never sit behind megabytes of stream
   descriptors (the HW drains the two queues round-robin).
 - Index/weight math on partition 0 in wide DVE ops; int16 indices are
   replicated to the 8 gpsimd core groups by a 0-stride broadcast DMA,
   lerp weights to 128 partitions the same way.
 - Per-chunk channel sums (ones-matmul into PSUM) right after each V slice;
   final cosine epilogue is one sqrt + one reciprocal on [1,384].
"""

import sys
from contextlib import ExitStack

import numpy as np

if "/opt/trn_rl_repo" not in sys.path:
    sys.path.insert(0, "/opt/trn_rl_repo")

B, N, C = 32, 32, 256
LEVELS = [(64, 64), (32, 32), (16, 16)]  # (H, W)
N_CORES = 8
BL = B // N_CORES          # images per core
NPTS = BL * N              # 128 points per core
PAIRS = [(0, 1), (0, 2), (1, 2)]
EPS = 1e-12

_CACHE = {}


def _build_program():
    from concourse import bacc, bass, mybir, tile, library_config

    dt = mybir.dt
    AL = mybir.AluOpType
    F32 = dt.float32
    I16 = dt.int16

    nc = bacc.Bacc("TRN2", target_bir_lowering=False, debug=False)

    feats = [
        nc.dram_tensor(f"feat{i}", [BL, C, H, W], F32, kind="ExternalInput")
        for i, (H, W) in enumerate(LEVELS)
    ]
    boxes = nc.dram_tensor("boxes", [BL, N, 4], F32, kind="ExternalInput")
    out = nc.dram_tensor("out", [1, 1], F32, kind="ExternalOutput")

    with tile.TileContext(nc) as tc, ExitStack() as ctx:
        pool = ctx.enter_context(tc.tile_pool(name="sbuf", bufs=1))
        pa = ctx.enter_context(tc.tile_pool(name="pa", bufs=1))
        pstream = ctx.enter_context(tc.tile_pool(name="stream", bufs=1))
        pwork = ctx.enter_context(tc.tile_pool(name="work", bufs=2))
        ppsum = ctx.enter_context(tc.tile_pool(name="psum", bufs=1, space="PSUM"))
        pdram = ctx.enter_context(tc.tile_pool(name="dram", bufs=1, space="DRAM"))

        nc.gpsimd.load_library(library_config.ap_gather)

        # warm-up: the first custom-op dispatch pays ~20us of Q7 ucode
        # installation; absorb it under the streaming with a dummy gather.
        dg_src = pool.tile([128, 4], F32, name="dg_src")
        nc.vector.memset(dg_src[:], 0.0)
        dg_idx = pool.tile([128, 1], I16, name="dg_idx")
        nc.vector.memset(dg_idx[:], 0)
        dg_out = pool.tile([128, 16], F32, name="dg_out")
        nc.gpsimd.ap_gather(
            out_ap=dg_out[:], in_ap=dg_src[:], idxs_ap=dg_idx[:],
            channels=128, num_elems=4, d=1, num_idxs=16,
        )

        # ---- boxes first on the sync queue ----
        bxr = pool.tile([1, BL * N * 4], F32)
        nc.sync.dma_start(
            out=bxr[:].rearrange("o (a f) -> o a f", a=BL),
            in_=boxes.rearrange("b n c -> b (n c)"),
        )

        # ---- stream tiles ----
        # l2/l1: [A_s0 | B_s0 | A_s1 | B_s1], each region nb*HW elements.
        T2 = pstream.tile([128, 4096], F32, name="T2")    # 16 KB/part
        T1 = pstream.tile([128, 16384], F32, name="T1")   # 64 KB/part
        # l0: per-image tiles [s0 | s1], 2-way rotating buffer (64 KB/part)
        T0 = [
            pstream.tile([128, 8192], F32, name=f"T0_{u}", tag="T0", bufs=2)
            for u in range(BL)
        ]

        # all A streams on the scalar queue, in gather order
        def fv(li):
            return feats[li].rearrange("b c h w -> c b (h w)")

        for li, T, n in ((2, T2, 1024), (1, T1, 4096)):
            for sec in range(2):
                nc.scalar.dma_start(
                    out=T[:, 2 * sec * n:(2 * sec + 1) * n].rearrange(
                        "c (b q) -> c b q", b=BL
                    ),
                    in_=fv(li)[sec * 128:sec * 128 + 128, :, :],
                )
        # l1 B region (shift-by-one) re-read from HBM per image; the last
        # element of each image's B row is never indexed - memset below.
        for sec in range(2):
            b0 = 2 * sec * 4096 + 4096
            bview = T1[:, b0:b0 + 4096].rearrange("c (b q) -> c b q", b=BL)
            nc.scalar.dma_start(
                out=bview[:, :, 0:1023],
                in_=fv(1)[sec * 128:sec * 128 + 128, :, 1:1024],
            )
            nc.vector.memset(bview[:, :, 1023:1024], 0.0)
        for u in range(BL):
            for sec in range(2):
                nc.scalar.dma_start(
                    out=T0[u][:, sec * 4096:(sec + 1) * 4096],
                    in_=fv(0)[sec * 128:sec * 128 + 128, u, :],
                )

        # B copies (shift-by-one) on the sync queue. l2's go out immediately;
        # l1's are emitted AFTER the index/weight staging DMAs so the staging
        # never queues behind 4 MB on the sync FIFO. The final element of
        # each B region is never indexed - memset it so the tile is fully
        # initialized for the d=2 gather's in_ap.
        def bcopy(T, n):
            dmas = []
            for sec in range(2):
                a0 = 2 * sec * n
                dmas.append(nc.sync.dma_start(
                    out=T[:, a0 + n:a0 + 2 * n - 1],
                    in_=T[:, a0 + 1:a0 + n],
                ))
                nc.vector.memset(T[:, a0 + 2 * n - 1:a0 + 2 * n], 0.0)
            return dmas

        bcopy(T2, 1024)
        # (l1's B region comes from HBM on the scalar queue instead - a 4 MB
        # SBUF copy here would either delay the staging DMAs in this FIFO or
        # be starved behind them.)

        # ---- Phase A: per-point scalar math on partition 0 (DVE) ----
        # segment layout on [1, 384]: cols li*128 + (b*32 + n), n = s*4 + rb
        SEG = lambda t, li: t[:, li * 128:(li + 1) * 128]

        CS = pa.tile([1, 384], F32, name="CS")       # E-1 per level
        for li, (H, W) in enumerate(LEVELS):
            nc.vector.memset(SEG(CS, li), float(W - 1))
        CE2 = pa.tile([1, 384], F32, name="CE2")     # E-2
        nc.vector.tensor_scalar(
            out=CE2[:], in0=CS[:], scalar1=-1.0, scalar2=None, op0=AL.add
        )
        CW = pa.tile([1, 384], F32, name="CW")       # E (row stride)
        nc.vector.tensor_scalar(
            out=CW[:], in0=CS[:], scalar1=1.0, scalar2=None, op0=AL.add
        )
        OFF = pa.tile([1, 384], F32, name="OFF")     # (b % nb) * HW
        nc.vector.memset(SEG(OFF, 0), 0.0)           # l0: nb=1
        for li in (1, 2):
            HW = LEVELS[li][0] * LEVELS[li][1]
            ov = SEG(OFF, li).rearrange("o (b n) -> o b n", b=BL)
            for b in range(BL):
                nc.vector.memset(ov[:, b], float(b * HW))

        bxv = bxr[:].rearrange("o (j c) -> o j c", c=4)

        # persistent per-axis outputs (slices written per pass)
        e0x = pa.tile([1, 384], F32, name="e0x")
        wex = pa.tile([1, 384], F32, name="wex")
        e0y = pa.tile([1, 384], F32, name="e0y")
        wey = pa.tile([1, 384], F32, name="wey")
        basef = pa.tile([1, 384], F32, name="basef")
        basei = pa.tile([1, 384], I16, name="basei")

        def axis_prep(coord, sl, e0t, wet, ax, tag):
            """p=clip(c*(E-1),0,E-1); e0=clamp(floor(p),0,E-2); w=p-e0.
            floor via 16.16 fixed point (exact *2^16). Operates on the
            [1, len(sl)] column slice sl."""
            w = sl.stop - sl.start
            pf = pa.tile([1, w], F32, name=f"pf{ax}{tag}", tag=f"tmp_pf{w}")
            for i in range(w // 128):
                nc.vector.tensor_tensor(
                    out=pf[:, i * 128:(i + 1) * 128], in0=coord,
                    in1=CS[:, sl.start + i * 128:sl.start + (i + 1) * 128],
                    op=AL.mult,
                )
            nc.vector.tensor_scalar_max(out=pf[:], in0=pf[:], scalar1=0.0)
            nc.vector.tensor_tensor(out=pf[:], in0=pf[:], in1=CS[:, sl], op=AL.min)
            pxs = pa.tile([1, w], F32, name=f"pxs{ax}{tag}", tag=f"tmp_pxs{w}")
            nc.vector.tensor_scalar(
                out=pxs[:], in0=pf[:], scalar1=65536.0, scalar2=None, op0=AL.mult
            )
            ifx = pa.tile([1, w], dt.int32, name=f"ifx{ax}{tag}",
                          tag=f"tmp_ifx{w}")
            nc.vector.tensor_copy(out=ifx[:], in_=pxs[:])
            nc.vector.tensor_scalar(
                out=ifx[:], in0=ifx[:], scalar1=16, scalar2=None,
                op0=AL.arith_shift_right,
            )
            e0 = e0t[:, sl]
            nc.vector.tensor_copy(out=e0, in_=ifx[:])
            nc.vector.tensor_tensor(out=e0, in0=e0, in1=CE2[:, sl], op=AL.min)
            nc.vector.tensor_tensor(out=wet[:, sl], in0=pf[:], in1=e0,
                                    op=AL.subtract)

        def base_prep(sl):
            bf = basef[:, sl]
            nc.vector.tensor_tensor(out=bf, in0=e0y[:, sl], in1=CW[:, sl],
                                    op=AL.mult)
            nc.vector.tensor_tensor(out=bf, in0=bf, in1=e0x[:, sl], op=AL.add)
            nc.vector.tensor_tensor(out=bf, in0=bf, in1=OFF[:, sl], op=AL.add)
            nc.vector.tensor_copy(out=basei[:, sl], in_=bf)

        # pass 1: level-2 segment only - puts srow's l2 columns (and with
        # them the first gather's indices) on the shortest possible path
        s2 = slice(256, 384)
        axis_prep(bxv[:, :, 0], s2, e0x, wex, "x", "2")
        axis_prep(bxv[:, :, 1], s2, e0y, wey, "y", "2")
        base_prep(s2)
        wx = wex
        wy = wey

        # ---- srow: wrapped int16 gather indices, [16 rows, 96 q-cols] ----
        # gather out col = q*16 + r; index stored at wrapped [r, q].
        # l2: q0=0,  l1: q0=32  (d=2 parity scheme):
        #   q = sec*16 + b*4 + s_hi, r = s_lo*8 + rb*2 + row
        #   p = base + row*W;  idx = (p>>1) + (p&1)*(nb*HW/2) + sec*(nb*HW)
        # l0: q0=64 (d=1 4-corner, one gather per image):
        #   q = 64 + u*16 + sec*8 + s, r = rb*4 + k
        #   idx = base + dk(k) + sec*4096
        srowA = pa.tile([1, 16 * 32], I16, name="srowA")   # l2 (32 q-cols)
        srowB = pa.tile([1, 16 * 96], I16, name="srowB")   # l1 + l0 (96)

        def srow_L(srow, li, q0):
            H, W = LEVELS[li]
            nbHW = BL * H * W
            for row in range(2):
                prow = pa.tile([1, 128], dt.int32, name=f"prow{li}{row}", tag="prow")
                nc.vector.tensor_scalar(
                    out=prow[:], in0=SEG(basei, li), scalar1=row * W,
                    scalar2=None, op0=AL.add,
                )
                par = pa.tile([1, 128], dt.int32, name=f"par{li}{row}", tag="par")
                nc.vector.tensor_scalar(
                    out=par[:], in0=prow[:], scalar1=1, scalar2=None,
                    op0=AL.bitwise_and,
                )
                nc.vector.tensor_scalar(
                    out=par[:], in0=par[:], scalar1=nbHW // 2, scalar2=None,
                    op0=AL.mult,
                )
                nc.vector.tensor_scalar(
                    out=prow[:], in0=prow[:], scalar1=1, scalar2=None,
                    op0=AL.arith_shift_right,
                )
                nc.vector.tensor_tensor(
                    out=prow[:], in0=prow[:], in1=par[:], op=AL.add
                )
                # scatter into srow sec0 cols: out[o, rb, b, s_hi] at
                # flat = (s_lo*8 + rb*2 + row)*QT + q0 + b*4 + s_hi
                sv = srow[:].rearrange(
                    "o (sl rb2 row2 q) -> o sl rb2 row2 q",
                    sl=2, rb2=4, row2=2,
                )  # r = s_lo*8 + rb*2 + row
                pv = prow[:].rearrange(
                    "o (b sh sl f) -> o b sh sl f", b=BL, sh=4, sl=2
                )  # col = b*32 + s_hi*8 + s_lo*4 + rb
                for s_lo in range(2):
                    otv = sv[:, s_lo, :, row, q0:q0 + 16].rearrange(
                        "o rb (b s) -> o rb b s", b=BL
                    )
                    itv = pv[:, :, :, s_lo, :].rearrange(
                        "o b sh f -> o f b sh"
                    )
                    nc.vector.tensor_copy(out=otv, in_=itv)
            # sec=1 cols = sec=0 cols + nb*HW
            sq = srow[:].rearrange("o (r q) -> o r q", r=16)
            nc.vector.tensor_scalar(
                out=sq[:, :, q0 + 16:q0 + 32], in0=sq[:, :, q0:q0 + 16],
                scalar1=nbHW, scalar2=None, op0=AL.add,
            )

        srow_L(srowA, 2, 0)

        # pass 2: l0+l1 segments of the axis chain
        s01 = slice(0, 256)
        axis_prep(bxv[:, :, 0], s01, e0x, wex, "x", "01")
        axis_prep(bxv[:, :, 1], s01, e0y, wey, "y", "01")
        base_prep(s01)
        srow_L(srowB, 1, 0)

        # l0 (d=1): per (u, k): out[o, rb, s] at r=rb*4+k (srowB),
        # q = 32 + u*16 + sec*8 + s
        sq = srowB[:].rearrange("o (rb k q) -> o rb k q", rb=4, k=4)
        sqr = srowB[:].rearrange("o (r q) -> o r q", r=16)
        b0 = SEG(basei, 0).rearrange("o (b s f) -> o f b s", b=BL, f=4)
        W0 = LEVELS[0][1]
        for u in range(BL):
            q0u = 32 + u * 16
            for k in range(4):
                dk = (k // 2) * W0 + (k % 2)
                nc.vector.tensor_scalar(
                    out=sq[:, :, k, q0u:q0u + 8],
                    in0=b0[:, :, u, :], scalar1=dk, scalar2=None, op0=AL.add,
                )
            nc.vector.tensor_scalar(
                out=sqr[:, :, q0u + 8:q0u + 16], in0=sqr[:, :, q0u:q0u + 8],
                scalar1=4096, scalar2=None, op0=AL.add,
            )

        # ---- replicate srow -> widx [128, 128] via DRAM round trips ----
        # two rounds: l2's 32 q-cols as soon as they exist (unblocks the
        # first gather ~8us earlier), then the remaining 96.
        widxA = pool.tile([128, 32], I16, name="widxA")   # l2
        widxB = pool.tile([128, 96], I16, name="widxB")   # l1 + l0
        sidx_d = pdram.tile([16, 32], I16, name="sidx")
        nc.sync.dma_start(
            out=sidx_d[:], in_=srowA[:].rearrange("o (r q) -> o r q", r=16)
        )
        nc.sync.dma_start(
            out=widxA[:],
            in_=sidx_d[:].unsqueeze(0).broadcast_to([8, 16, 32]),
        )
        sidx2_d = pdram.tile([16, 96], I16, name="sidx2")
        nc.sync.dma_start(
            out=sidx2_d[:], in_=srowB[:].rearrange("o (r q) -> o r q", r=16)
        )
        nc.sync.dma_start(
            out=widxB[:],
            in_=sidx2_d[:].unsqueeze(0).broadcast_to([8, 16, 96]),
        )

        # ---- lerp weights wrow [1, 1536] -> wb [128, 1536] ----
        # l2: 0:512, l1: 512:1024  (cols (b, s, rb, row, j) = pt*4+row*2+j,
        #   weight = yw(row) * xw(j))
        # l0: 1024:1536 (cols (u, s, rb, k) = pt*4+k, weight = yw(k)*xw(k))
        w1x = pa.tile([1, 384], F32, name="w1x")
        nc.vector.tensor_scalar(
            out=w1x[:], in0=wx[:], scalar1=-1.0, scalar2=1.0,
            op0=AL.mult, op1=AL.add,
        )
        w1y = pa.tile([1, 384], F32, name="w1y")
        nc.vector.tensor_scalar(
            out=w1y[:], in0=wy[:], scalar1=-1.0, scalar2=1.0,
            op0=AL.mult, op1=AL.add,
        )
        wrow = pa.tile([1, 1536], F32, name="wrow")
        iv = lambda t, li: SEG(t, li).rearrange(
            "o (b s f) -> o b s f", b=BL, f=4
        )
        for li, w0 in ((2, 0), (1, 512)):
            wseg = wrow[:, w0:w0 + 512].rearrange(
                "o (b s f row j) -> o b s f row j", b=BL, s=8, f=4, row=2
            )
            for row, ywt in ((0, w1y), (1, wy)):
                for j, xwt in ((0, w1x), (1, wx)):
                    nc.vector.tensor_tensor(
                        out=wseg[:, :, :, :, row, j],
                        in0=iv(ywt, li), in1=iv(xwt, li), op=AL.mult,
                    )
        wseg = wrow[:, 1024:1536].rearrange(
            "o (b s f k) -> o b s f k", b=BL, s=8, f=4
        )
        for k, (ywt, xwt) in enumerate(
            [(w1y, w1x), (w1y, wx), (wy, w1x), (wy, wx)]
        ):
            nc.vector.tensor_tensor(
                out=wseg[:, :, :, :, k], in0=iv(ywt, 0), in1=iv(xwt, 0),
                op=AL.mult,
            )
        wrow_d = pdram.tile([1, 1536], F32, name="wrow_d")
        nc.sync.dma_start(out=wrow_d[:], in_=wrow[:])
        wb = pool.tile([128, 1536], F32, name="wb")
        nc.sync.dma_start(out=wb[:], in_=wrow_d[:].broadcast_to([128, 1536]))

        # ---- gathers + lerp + reduce + per-chunk channel sums ----
        V = pool.tile([128, 768], F32, name="V")
        ones = pool.tile([128, 1], F32)
        nc.vector.memset(ones[:], 1.0)

        ps_ss = ppsum.tile([1, 512], F32, name="ps_ss")    # ss2 | ss1
        ps_ss0 = ppsum.tile([1, 256], F32, name="ps_ss0")  # (u, sec, n)
        ps_d12 = ppsum.tile([1, 256], F32, name="ps_d12")
        ps_d01 = ppsum.tile([1, 256], F32, name="ps_d01")
        ps_d02 = ppsum.tile([1, 256], F32, name="ps_d02")

        def colsum(ps_slice, in0, in1, n, tag):
            prod = pwork.tile([128, 256], F32, name=f"prod{tag}", tag="prod",
                              bufs=1)
            nc.vector.tensor_tensor(
                out=prod[:, 0:n], in0=in0, in1=in1, op=AL.mult
            )
            nc.tensor.matmul(
                ps_slice, ones[:], prod[:, 0:n], start=True, stop=True
            )

        def gatherL(li, T, q0, v0, w0, tag):
            # l2/l1: one d=2 gather, og [128, 1024]
            og = pwork.tile([128, 1024], F32, name=f"og{tag}", tag="ogL",
                            bufs=1)
            nc.gpsimd.ap_gather(
                out_ap=og[:],
                in_ap=T[:].rearrange("c (n e) -> c n e", e=2),
                idxs_ap=widxA[:] if q0 == 0 else widxB[:, 0:32],
                channels=128, num_elems=T.shape[1] // 2, d=2, num_idxs=512,
            )
            for sec in range(2):
                nc.vector.tensor_tensor(
                    out=og[:, sec * 512:(sec + 1) * 512],
                    in0=og[:, sec * 512:(sec + 1) * 512],
                    in1=wb[:, w0:w0 + 512], op=AL.mult,
                )
            nc.vector.tensor_reduce(
                out=V[:, v0:v0 + 256],
                in_=og[:].rearrange("c (n f) -> c n f", f=4),
                axis=mybir.AxisListType.X, op=AL.add,
            )
            colsum(ps_ss[:, v0:v0 + 256], V[:, v0:v0 + 256],
                   V[:, v0:v0 + 256], 256, f"ss{tag}")

        def gather0(u):
            # l0: one 256-idx d=1 gather per image, og [128, 256], + ss0
            og = pwork.tile([128, 256], F32, name=f"og0{u}", tag="og0",
                            bufs=1)
            nc.gpsimd.ap_gather(
                out_ap=og[:], in_ap=T0[u][:],
                idxs_ap=widxB[:, 32 + u * 16:32 + u * 16 + 16],
                channels=128, num_elems=8192, d=1, num_idxs=256,
            )
            for sec in range(2):
                nc.vector.tensor_tensor(
                    out=og[:, sec * 128:(sec + 1) * 128],
                    in0=og[:, sec * 128:(sec + 1) * 128],
                    in1=wb[:, 1024 + u * 128:1024 + (u + 1) * 128],
                    op=AL.mult,
                )
            v0 = 512 + u * 64
            nc.vector.tensor_reduce(
                out=V[:, v0:v0 + 64],
                in_=og[:].rearrange("c (n f) -> c n f", f=4),
                axis=mybir.AxisListType.X, op=AL.add,
            )
            v0u = V[:, v0:v0 + 64]
            colsum(ps_ss0[:, u * 64:(u + 1) * 64], v0u, v0u, 64, f"ss0{u}")

        def dots0(u):
            # cross-level dots for image u (needs V1/V2 slices emitted)
            v0u = V[:, 512 + 64 * u:512 + 64 * (u + 1)]
            v1u = V[:, 256:512].rearrange(
                "c (sec b n) -> c sec b n", sec=2, b=BL
            )[:, :, u, :]
            v2u = V[:, 0:256].rearrange(
                "c (sec b n) -> c sec b n", sec=2, b=BL
            )[:, :, u, :]
            sl = slice(u * 64, (u + 1) * 64)
            colsum(ps_d01[:, sl], v0u, v1u, 64, f"d01{u}")
            colsum(ps_d02[:, sl], v0u, v2u, 64, f"d02{u}")

        gatherL(2, T2, 0, 0, 0, "2")
        gatherL(1, T1, 32, 256, 512, "1")
        colsum(ps_d12[:], V[:, 256:512], V[:, 0:256], 256, "d12")
        for u in range(BL):
            gather0(u)
            dots0(u)

        # ---- epilogue on partition 0 ----
        cs_ss = pool.tile([1, 512], F32, name="cs_ss")
        nc.vector.tensor_copy(out=cs_ss[:], in_=ps_ss[:])
        cs_ss0 = pool.tile([1, 256], F32, name="cs_ss0")
        nc.vector.tensor_copy(out=cs_ss0[:], in_=ps_ss0[:])
        cs_d01 = pool.tile([1, 256], F32, name="cs_d01")
        nc.vector.tensor_copy(out=cs_d01[:], in_=ps_d01[:])
        cs_d02 = pool.tile([1, 256], F32, name="cs_d02")
        nc.vector.tensor_copy(out=cs_d02[:], in_=ps_d02[:])
        cs_d12 = pool.tile([1, 256], F32, name="cs_d12")
        nc.vector.tensor_copy(out=cs_d12[:], in_=ps_d12[:])

        ssc = pool.tile([1, 384], F32, name="ssc")
        dc = pool.tile([1, 384], F32, name="dc")

        def secsum(dst, src, l0_layout):
            if l0_layout:  # src [1, 256] cols (u, sec, n)
                v = src.rearrange("o (u sec n) -> o u sec n", u=BL, sec=2)
                nc.vector.tensor_tensor(
                    out=dst.rearrange("o (u n) -> o u n", u=BL),
                    in0=v[:, :, 0], in1=v[:, :, 1], op=AL.add,
                )
            else:  # src [1, 256] cols (sec, b, n)
                nc.vector.tensor_tensor(
                    out=dst, in0=src[:, 0:128], in1=src[:, 128:256], op=AL.add
                )

        secsum(SEG(ssc, 0), cs_ss0[:], True)
        secsum(SEG(ssc, 1), cs_ss[:, 256:512], False)
        secsum(SEG(ssc, 2), cs_ss[:, 0:256], False)
        secsum(SEG(dc, 0), cs_d01[:], True)
        secsum(SEG(dc, 1), cs_d02[:], True)
        secsum(SEG(dc, 2), cs_d12[:], False)

        # rn = 1 / max(sqrt(ssc), EPS) == 1 / sqrt(max(ssc, EPS^2))
        nc.vector.tensor_scalar_max(out=ssc[:], in0=ssc[:], scalar1=EPS * EPS)
        nrm = pool.tile([1, 384], F32, name="nrm")
        nc.scalar.sqrt(out=nrm[:], in_=ssc[:])
        rn = pool.tile([1, 384], F32, name="rn")
        nc.vector.reciprocal(out=rn[:], in_=nrm[:])

        rp = pool.tile([1, 384], F32, name="rp")
        for seg, (i, j) in enumerate(PAIRS):
            nc.vector.tensor_tensor(
                out=SEG(rp, seg), in0=SEG(rn, i), in1=SEG(rn, j), op=AL.mult
            )
        nc.vector.tensor_tensor(out=dc[:], in0=dc[:], in1=rp[:], op=AL.mult)
        res = pool.tile([1, 1], F32)
        nc.vector.tensor_reduce(
            out=res[:], in_=dc[:], axis=mybir.AxisListType.X, op=AL.add
        )
        nc.sync.dma_start(out=out.ap(), in_=res[:])

    nc.compile()
    return nc


def _get_program():
    if "nc" not in _CACHE:
        _CACHE["nc"] = _build_program()
    return _CACHE["nc"]


def _run_device(feat0, feat1, feat2, boxes, **run_kwargs):
    from concourse.bass_utils import run_bass_kernel_spmd

    nc = _get_program()

    feats = [
        np.ascontiguousarray(np.asarray(f, dtype=np.float32))
        for f in (feat0, feat1, feat2)
    ]
    boxes = np.ascontiguousarray(np.asarray(boxes, dtype=np.float32))

    in_maps = []
    for k in range(N_CORES):
        sl = slice(k * BL, (k + 1) * BL)
        in_maps.append(
            {
                "feat0": feats[0][sl],
                "feat1": feats[1][sl],
                "feat2": feats[2][sl],
                "boxes": boxes[sl],
            }
        )

    return run_bass_kernel_spmd(
        nc, in_maps, core_ids=list(range(N_CORES)), **run_kwargs
    )


def kernel(feat0, feat1, feat2, boxes):
    r = _run_device(feat0, feat1, feat2, boxes)
    total = np.float64(0.0)
    for m in r.results:
        total += np.float64(m["out"].reshape(-1)[0])

    count = B * N * len(PAIRS)
    avg = np.float32(total) / np.float32(count)
    loss = np.float32(1.0) - avg
    loss = np.nan_to_num(loss, nan=0.0, posinf=1.0, neginf=0.0)
    return np.array(np.clip(loss, 0.0, 2.0), dtype=np.float32)


# revision 12
# speedup vs baseline: 1.0474x; 1.0474x over previous
"""Trainium2 Bass kernel for nn_CSCLoss: multi-scale bilinear point-sampling
cosine-consistency loss.

loss = 1 - mean_{pairs,(b,n)} <normalize(sample(feat_i, p_bn)), normalize(sample(feat_j, p_bn))>

Sharding: data-parallel over batch - 32 images -> 8 cores x 4 images; the
host sums the 8 per-core partial sums and applies the loss epilogue.

Per-core dataflow. ap_gather cost is ~27ns per INDEX nearly independent of
d (measured), so the design minimizes index count:
 - l2/l1 stream into tiles holding [A_s0|B_s0|A_s1|B_s1] where B = A shifted
   by one element (built by SBUF->SBUF DMA, no extra HBM). Any x-pair
   (p, p+1) is then an even-aligned d=2 block: of A if p is even, of B if p
   is odd. One gather index per (point, row, channel-chunk): 512 idx/level
   instead of 1024.
 - l0 (16.8 MB/core) keeps d=1 4-corner gathers (a B copy would cost too
   much SBUF/DMA): 8 per-(image,chunk) tiles of [128,4096], 1024 idx total.
 - All 12 stream DMAs ride the scalar HWDGE queue in arrival order; the
   sync queue carries only boxes, the 4 B-copies, index/weight staging and
   the result, so small transfers never sit behind megabytes of stream
   descriptors (the HW drains the two queues round-robin).
 - Index/weight math on partition 0 in wide DVE ops; int16 indices are
   replicated to the 8 gpsimd core groups by a 0-stride broadcast DMA,
   lerp weights to 128 partitions the same way.
 - Per-chunk channel sums (ones-matmul into PSUM) right after each V slice;
   final cosine epilogue is one sqrt + one reciprocal on [1,384].
"""

import sys
from contextlib import ExitStack

import numpy as np

if "/opt/trn_rl_repo" not in sys.path:
    sys.path.insert(0, "/opt/trn_rl_repo")

B, N, C = 32, 32, 256
LEVELS = [(64, 64), (32, 32), (16, 16)]  # (H, W)
N_CORES = 8
BL = B // N_CORES          # images per core
NPTS = BL * N              # 128 points per core
PAIRS = [(0, 1), (0, 2), (1, 2)]
EPS = 1e-12

_CACHE = {}


def _build_program():
    from concourse import bacc, bass, mybir, tile, library_config

    dt = mybir.dt
    AL = mybir.AluOpType
    F32 = dt.float32
    I16 = dt.int16

    nc = bacc.Bacc("TRN2", target_bir_lowering=False, debug=False)

    feats = [
        nc.dram_tensor(f"feat{i}", [BL, C, H, W], F32, kind="ExternalInput")
        for i, (H, W) in enumerate(LEVELS)
    ]
    boxes = nc.dram_tensor("boxes", [BL, N, 4], F32, kind="ExternalInput")
    out = nc.dram_tensor("out", [1, 1], F32, kind="ExternalOutput")

    with tile.TileContext(nc) as tc, ExitStack() as ctx:
        pool = ctx.enter_context(tc.tile_pool(name="sbuf", bufs=1))
        pa = ctx.enter_context(tc.tile_pool(name="pa", bufs=1))
        pstream = ctx.enter_context(tc.tile_pool(name="stream", bufs=1))
        pwork = ctx.enter_context(tc.tile_pool(name="work", bufs=2))
        ppsum = ctx.enter_context(tc.tile_pool(name="psum", bufs=1, space="PSUM"))
        pdram = ctx.enter_context(tc.tile_pool(name="dram", bufs=1, space="DRAM"))

        nc.gpsimd.load_library(library_config.ap_gather)

        # warm-up: the first custom-op dispatch pays ~20us of Q7 ucode
        # installation; absorb it under the streaming with a dummy gather.
        dg_src = pool.tile([128, 4], F32, name="dg_src")
        nc.vector.memset(dg_src[:], 0.0)
        dg_idx = pool.tile([128, 1], I16, name="dg_idx")
        nc.vector.memset(dg_idx[:], 0)
        dg_out = pool.tile([128, 16], F32, name="dg_out")
        nc.gpsimd.ap_gather(
            out_ap=dg_out[:], in_ap=dg_src[:], idxs_ap=dg_idx[:],
            channels=128, num_elems=4, d=1, num_idxs=16,
        )

        # ---- boxes first on the sync queue ----
        bxr = pool.tile([1, BL * N * 4], F32)
        nc.sync.dma_start(
            out=bxr[:].rearrange("o (a f) -> o a f", a=BL),
            in_=boxes.rearrange("b n c -> b (n c)"),
        )

        # ---- stream tiles ----
        # l2/l1: [A_s0 | B_s0 | A_s1 | B_s1], each region nb*HW elements.
        T2 = pstream.tile([128, 4096], F32, name="T2")    # 16 KB/part
        T1 = pstream.tile([128, 16384], F32, name="T1")   # 64 KB/part
        # l0: per-image tiles [s0 | s1], 2-way rotating buffer (64 KB/part)
        T0 = [
            pstream.tile([128, 8192], F32, name=f"T0_{u}", tag="T0", bufs=2)
            for u in range(BL)
        ]

        # all A streams on the scalar queue, in gather order
        def fv(li):
            return feats[li].rearrange("b c h w -> c b (h w)")

        for li, T, n in ((2, T2, 1024), (1, T1, 4096)):
            for sec in range(2):
                nc.scalar.dma_start(
                    out=T[:, 2 * sec * n:(2 * sec + 1) * n].rearrange(
                        "c (b q) -> c b q", b=BL
                    ),
                    in_=fv(li)[sec * 128:sec * 128 + 128, :, :],
                )
        # l1 B region (shift-by-one) re-read from HBM per image; the last
        # element of each image's B row is never indexed - memset below.
        for sec in range(2):
            b0 = 2 * sec * 4096 + 4096
            bview = T1[:, b0:b0 + 4096].rearrange("c (b q) -> c b q", b=BL)
            nc.scalar.dma_start(
                out=bview[:, :, 0:1023],
                in_=fv(1)[sec * 128:sec * 128 + 128, :, 1:1024],
            )
            nc.vector.memset(bview[:, :, 1023:1024], 0.0)
        for u in range(BL):
            for sec in range(2):
                nc.scalar.dma_start(
                    out=T0[u][:, sec * 4096:(sec + 1) * 4096],
                    in_=fv(0)[sec * 128:sec * 128 + 128, u, :],
                )

        # B copies (shift-by-one) on the sync queue. l2's go out immediately;
        # l1's are emitted AFTER the index/weight staging DMAs so the staging
        # never queues behind 4 MB on the sync FIFO. The final element of
        # each B region is never indexed - memset it so the tile is fully
        # initialized for the d=2 gather's in_ap.
        def bcopy(T, n):
            dmas = []
            for sec in range(2):
                a0 = 2 * sec * n
                dmas.append(nc.sync.dma_start(
                    out=T[:, a0 + n:a0 + 2 * n - 1],
                    in_=T[:, a0 + 1:a0 + n],
                ))
                nc.vector.memset(T[:, a0 + 2 * n - 1:a0 + 2 * n], 0.0)
            return dmas

        bcopy(T2, 1024)
        # (l1's B region comes from HBM on the scalar queue instead - a 4 MB
        # SBUF copy here would either delay the staging DMAs in this FIFO or
        # be starved behind them.)

        # ---- Phase A: per-point scalar math on partition 0 (DVE) ----
        # segment layout on [1, 384]: cols li*128 + (b*32 + n), n = s*4 + rb
        SEG = lambda t, li: t[:, li * 128:(li + 1) * 128]

        CS = pa.tile([1, 384], F32, name="CS")       # E-1 per level
        for li, (H, W) in enumerate(LEVELS):
            nc.vector.memset(SEG(CS, li), float(W - 1))
        OFF = pa.tile([1, 256], F32, name="OFF")     # (b%nb)*HW for l1, l2
        for li in (1, 2):
            HW = LEVELS[li][0] * LEVELS[li][1]
            ov = OFF[:, (li - 1) * 128:li * 128].rearrange(
                "o (b n) -> o b n", b=BL
            )
            for b in range(BL):
                nc.vector.memset(ov[:, b], float(b * HW))

        bxv = bxr[:].rearrange("o (j c) -> o j c", c=4)

        # persistent per-axis outputs (slices written per pass)
        e0x = pa.tile([1, 384], F32, name="e0x")
        wex = pa.tile([1, 384], F32, name="wex")
        e0y = pa.tile([1, 384], F32, name="e0y")
        wey = pa.tile([1, 384], F32, name="wey")
        basef = pa.tile([1, 384], F32, name="basef")
        basei = pa.tile([1, 384], I16, name="basei")

        def axis_prep(coord, sl, e0t, wet, ax, tag):
            """p=clip(c*(E-1),0,E-1); e0=clamp(floor(p),0,E-2); w=p-e0.
            floor via 16.16 fixed point (exact *2^16). Operates on the
            [1, len(sl)] column slice sl."""
            w = sl.stop - sl.start
            pf = pa.tile([1, w], F32, name=f"pf{ax}{tag}", tag=f"tmp_pf{w}")
            for i in range(w // 128):
                nc.vector.tensor_tensor(
                    out=pf[:, i * 128:(i + 1) * 128], in0=coord,
                    in1=CS[:, sl.start + i * 128:sl.start + (i + 1) * 128],
                    op=AL.mult,
                )
            nc.vector.tensor_scalar_max(out=pf[:], in0=pf[:], scalar1=0.0)
            nc.vector.tensor_tensor(out=pf[:], in0=pf[:], in1=CS[:, sl], op=AL.min)
            pxs = pa.tile([1, w], F32, name=f"pxs{ax}{tag}", tag=f"tmp_pxs{w}")
            nc.vector.tensor_scalar(
                out=pxs[:], in0=pf[:], scalar1=65536.0, scalar2=None, op0=AL.mult
            )
            ifx = pa.tile([1, w], dt.int32, name=f"ifx{ax}{tag}",
                          tag=f"tmp_ifx{w}")
            nc.vector.tensor_copy(out=ifx[:], in_=pxs[:])
            nc.vector.tensor_scalar(
                out=ifx[:], in0=ifx[:], scalar1=16, scalar2=None,
                op0=AL.arith_shift_right,
            )
            e0 = e0t[:, sl]
            nc.vector.tensor_copy(out=e0, in_=ifx[:])
            for i in range(w // 128):
                li = (sl.start + i * 128) // 128
                nc.vector.tensor_scalar_min(
                    out=e0t[:, sl.start + i * 128:sl.start + (i + 1) * 128],
                    in0=e0t[:, sl.start + i * 128:sl.start + (i + 1) * 128],
                    scalar1=float(LEVELS[li][1] - 2),
                )
            nc.vector.tensor_tensor(out=wet[:, sl], in0=pf[:], in1=e0,
                                    op=AL.subtract)

        def base_prep(sl):
            for i in range((sl.stop - sl.start) // 128):
                li = (sl.start + i * 128) // 128
                s1 = slice(sl.start + i * 128, sl.start + (i + 1) * 128)
                bf = basef[:, s1]
                nc.vector.tensor_scalar(
                    out=bf, in0=e0y[:, s1], scalar1=float(LEVELS[li][1]),
                    scalar2=None, op0=AL.mult,
                )
                nc.vector.tensor_tensor(out=bf, in0=bf, in1=e0x[:, s1],
                                        op=AL.add)
                if li > 0:
                    nc.vector.tensor_tensor(
                        out=bf, in0=bf,
                        in1=OFF[:, (li - 1) * 128:li * 128], op=AL.add,
                    )
                nc.vector.tensor_copy(out=basei[:, s1], in_=bf)

        # pass 1: level-2 segment only - puts srow's l2 columns (and with
        # them the first gather's indices) on the shortest possible path
        s2 = slice(256, 384)
        axis_prep(bxv[:, :, 0], s2, e0x, wex, "x", "2")
        axis_prep(bxv[:, :, 1], s2, e0y, wey, "y", "2")
        base_prep(s2)
        wx = wex
        wy = wey

        # ---- srow: wrapped int16 gather indices, [16 rows, 96 q-cols] ----
        # gather out col = q*16 + r; index stored at wrapped [r, q].
        # l2: q0=0,  l1: q0=32  (d=2 parity scheme):
        #   q = sec*16 + b*4 + s_hi, r = s_lo*8 + rb*2 + row
        #   p = base + row*W;  idx = (p>>1) + (p&1)*(nb*HW/2) + sec*(nb*HW)
        # l0: q0=64 (d=1 4-corner, one gather per image):
        #   q = 64 + u*16 + sec*8 + s, r = rb*4 + k
        #   idx = base + dk(k) + sec*4096
        srowA = pa.tile([1, 16 * 32], I16, name="srowA")   # l2 (32 q-cols)
        srowB = pa.tile([1, 16 * 96], I16, name="srowB")   # l1 + l0 (96)

        def srow_L(srow, li, q0):
            H, W = LEVELS[li]
            nbHW = BL * H * W
            for row in range(2):
                prow = pa.tile([1, 128], dt.int32, name=f"prow{li}{row}", tag="prow")
                nc.vector.tensor_scalar(
                    out=prow[:], in0=SEG(basei, li), scalar1=row * W,
                    scalar2=None, op0=AL.add,
                )
                par = pa.tile([1, 128], dt.int32, name=f"par{li}{row}", tag="par")
                nc.vector.tensor_scalar(
                    out=par[:], in0=prow[:], scalar1=1, scalar2=None,
                    op0=AL.bitwise_and,
                )
                nc.vector.tensor_scalar(
                    out=par[:], in0=par[:], scalar1=nbHW // 2, scalar2=None,
                    op0=AL.mult,
                )
                nc.vector.tensor_scalar(
                    out=prow[:], in0=prow[:], scalar1=1, scalar2=None,
                    op0=AL.arith_shift_right,
                )
                nc.vector.tensor_tensor(
                    out=prow[:], in0=prow[:], in1=par[:], op=AL.add
                )
                # scatter into srow sec0 cols: out[o, rb, b, s_hi] at
                # flat = (s_lo*8 + rb*2 + row)*QT + q0 + b*4 + s_hi
                sv = srow[:].rearrange(
                    "o (sl rb2 row2 q) -> o sl rb2 row2 q",
                    sl=2, rb2=4, row2=2,
                )  # r = s_lo*8 + rb*2 + row
                pv = prow[:].rearrange(
                    "o (b sh sl f) -> o b sh sl f", b=BL, sh=4, sl=2
                )  # col = b*32 + s_hi*8 + s_lo*4 + rb
                for s_lo in range(2):
                    otv = sv[:, s_lo, :, row, q0:q0 + 16].rearrange(
                        "o rb (b s) -> o rb b s", b=BL
                    )
                    itv = pv[:, :, :, s_lo, :].rearrange(
                        "o b sh f -> o f b sh"
                    )
                    nc.vector.tensor_copy(out=otv, in_=itv)
            # sec=1 cols = sec=0 cols + nb*HW
            sq = srow[:].rearrange("o (r q) -> o r q", r=16)
            nc.vector.tensor_scalar(
                out=sq[:, :, q0 + 16:q0 + 32], in0=sq[:, :, q0:q0 + 16],
                scalar1=nbHW, scalar2=None, op0=AL.add,
            )

        srow_L(srowA, 2, 0)

        # pass 2: l0+l1 segments of the axis chain
        s01 = slice(0, 256)
        axis_prep(bxv[:, :, 0], s01, e0x, wex, "x", "01")
        axis_prep(bxv[:, :, 1], s01, e0y, wey, "y", "01")
        base_prep(s01)
        srow_L(srowB, 1, 0)

        # l0 (d=1): per (u, k): out[o, rb, s] at r=rb*4+k (srowB),
        # q = 32 + u*16 + sec*8 + s
        sq = srowB[:].rearrange("o (rb k q) -> o rb k q", rb=4, k=4)
        sqr = srowB[:].rearrange("o (r q) -> o r q", r=16)
        b0 = SEG(basei, 0).rearrange("o (b s f) -> o f b s", b=BL, f=4)
        W0 = LEVELS[0][1]
        for u in range(BL):
            q0u = 32 + u * 16
            for k in range(4):
                dk = (k // 2) * W0 + (k % 2)
                nc.vector.tensor_scalar(
                    out=sq[:, :, k, q0u:q0u + 8],
                    in0=b0[:, :, u, :], scalar1=dk, scalar2=None, op0=AL.add,
                )
            nc.vector.tensor_scalar(
                out=sqr[:, :, q0u + 8:q0u + 16], in0=sqr[:, :, q0u:q0u + 8],
                scalar1=4096, scalar2=None, op0=AL.add,
            )

        # ---- replicate srow -> widx [128, 128] via DRAM round trips ----
        # two rounds: l2's 32 q-cols as soon as they exist (unblocks the
        # first gather ~8us earlier), then the remaining 96.
        # index replication rides the gpsimd SWDGE queue: its completion
        # semaphores (DMASW lanes) are private to gpsimd DMAs, so the gather
        # FIFO never inherits false waits on fat stream DMAs sharing a
        # DMAHW lane. widxA (l2) lands before g2; the B round trip is
        # emitted between g2 and g1 (see below).
        widxA = pool.tile([128, 32], I16, name="widxA")   # l2
        widxB = pool.tile([128, 96], I16, name="widxB")   # l1 + l0
        sidx_d = pdram.tile([16, 32], I16, name="sidx")
        nc.gpsimd.dma_start(
            out=sidx_d[:], in_=srowA[:].rearrange("o (r q) -> o r q", r=16)
        )
        nc.gpsimd.dma_start(
            out=widxA[:],
            in_=sidx_d[:].unsqueeze(0).broadcast_to([8, 16, 32]),
        )

        def stage_B():
            sidx2_d = pdram.tile([16, 96], I16, name="sidx2")
            nc.gpsimd.dma_start(
                out=sidx2_d[:],
                in_=srowB[:].rearrange("o (r q) -> o r q", r=16),
            )
            nc.gpsimd.dma_start(
                out=widxB[:],
                in_=sidx2_d[:].unsqueeze(0).broadcast_to([8, 16, 96]),
            )

        # ---- lerp weights wrow [1, 1536] -> wb [128, 1536] ----
        # l2: 0:512, l1: 512:1024  (cols (b, s, rb, row, j) = pt*4+row*2+j,
        #   weight = yw(row) * xw(j))
        # l0: 1024:1536 (cols (u, s, rb, k) = pt*4+k, weight = yw(k)*xw(k))
        w1x = pa.tile([1, 384], F32, name="w1x")
        nc.vector.tensor_scalar(
            out=w1x[:], in0=wx[:], scalar1=-1.0, scalar2=1.0,
            op0=AL.mult, op1=AL.add,
        )
        w1y = pa.tile([1, 384], F32, name="w1y")
        nc.vector.tensor_scalar(
            out=w1y[:], in0=wy[:], scalar1=-1.0, scalar2=1.0,
            op0=AL.mult, op1=AL.add,
        )
        wrow = pa.tile([1, 1536], F32, name="wrow")
        iv = lambda t, li: SEG(t, li).rearrange(
            "o (b s f) -> o b s f", b=BL, f=4
        )
        for li, w0 in ((2, 0), (1, 512)):
            wseg = wrow[:, w0:w0 + 512].rearrange(
                "o (b s f row j) -> o b s f row j", b=BL, s=8, f=4, row=2
            )
            for row, ywt in ((0, w1y), (1, wy)):
                for j, xwt in ((0, w1x), (1, wx)):
                    nc.vector.tensor_tensor(
                        out=wseg[:, :, :, :, row, j],
                        in0=iv(ywt, li), in1=iv(xwt, li), op=AL.mult,
                    )
        wseg = wrow[:, 1024:1536].rearrange(
            "o (b s f k) -> o b s f k", b=BL, s=8, f=4
        )
        for k, (ywt, xwt) in enumerate(
            [(w1y, w1x), (w1y, wx), (wy, w1x), (wy, wx)]
        ):
            nc.vector.tensor_tensor(
                out=wseg[:, :, :, :, k], in0=iv(ywt, 0), in1=iv(xwt, 0),
                op=AL.mult,
            )
        wrow_d = pdram.tile([1, 1536], F32, name="wrow_d")
        nc.sync.dma_start(out=wrow_d[:], in_=wrow[:])
        wb = pool.tile([128, 1536], F32, name="wb")
        nc.sync.dma_start(out=wb[:], in_=wrow_d[:].broadcast_to([128, 1536]))

        # ---- gathers + lerp + reduce + per-chunk channel sums ----
        V = pool.tile([128, 768], F32, name="V")
        ones = pool.tile([128, 1], F32)
        nc.vector.memset(ones[:], 1.0)

        ps_ss = ppsum.tile([1, 512], F32, name="ps_ss")    # ss2 | ss1
        ps_ss0 = ppsum.tile([1, 256], F32, name="ps_ss0")  # (u, sec, n)
        ps_d12 = ppsum.tile([1, 256], F32, name="ps_d12")
        ps_d01 = ppsum.tile([1, 256], F32, name="ps_d01")
        ps_d02 = ppsum.tile([1, 256], F32, name="ps_d02")

        def colsum(ps_slice, in0, in1, n, tag):
            prod = pwork.tile([128, 256], F32, name=f"prod{tag}", tag="prod",
                              bufs=1)
            nc.vector.tensor_tensor(
                out=prod[:, 0:n], in0=in0, in1=in1, op=AL.mult
            )
            nc.tensor.matmul(
                ps_slice, ones[:], prod[:, 0:n], start=True, stop=True
            )

        def gatherL(li, T, q0, v0, w0, tag):
            # l2/l1: one d=2 gather, og [128, 1024]
            og = pwork.tile([128, 1024], F32, name=f"og{tag}", tag="ogL",
                            bufs=2)
            nc.gpsimd.ap_gather(
                out_ap=og[:],
                in_ap=T[:].rearrange("c (n e) -> c n e", e=2),
                idxs_ap=widxA[:] if q0 == 0 else widxB[:, 0:32],
                channels=128, num_elems=T.shape[1] // 2, d=2, num_idxs=512,
            )
            for sec in range(2):
                nc.vector.tensor_tensor(
                    out=og[:, sec * 512:(sec + 1) * 512],
                    in0=og[:, sec * 512:(sec + 1) * 512],
                    in1=wb[:, w0:w0 + 512], op=AL.mult,
                )
            nc.vector.tensor_reduce(
                out=V[:, v0:v0 + 256],
                in_=og[:].rearrange("c (n f) -> c n f", f=4),
                axis=mybir.AxisListType.X, op=AL.add,
            )
            colsum(ps_ss[:, v0:v0 + 256], V[:, v0:v0 + 256],
                   V[:, v0:v0 + 256], 256, f"ss{tag}")

        def gather0(u):
            # l0: one 256-idx d=1 gather per image, og [128, 256], + ss0
            og = pwork.tile([128, 256], F32, name=f"og0{u}", tag="og0",
                            bufs=1)
            nc.gpsimd.ap_gather(
                out_ap=og[:], in_ap=T0[u][:],
                idxs_ap=widxB[:, 32 + u * 16:32 + u * 16 + 16],
                channels=128, num_elems=8192, d=1, num_idxs=256,
            )
            for sec in range(2):
                nc.vector.tensor_tensor(
                    out=og[:, sec * 128:(sec + 1) * 128],
                    in0=og[:, sec * 128:(sec + 1) * 128],
                    in1=wb[:, 1024 + u * 128:1024 + (u + 1) * 128],
                    op=AL.mult,
                )
            v0 = 512 + u * 64
            nc.vector.tensor_reduce(
                out=V[:, v0:v0 + 64],
                in_=og[:].rearrange("c (n f) -> c n f", f=4),
                axis=mybir.AxisListType.X, op=AL.add,
            )
            v0u = V[:, v0:v0 + 64]
            colsum(ps_ss0[:, u * 64:(u + 1) * 64], v0u, v0u, 64, f"ss0{u}")

        def dots0(u):
            # cross-level dots for image u (needs V1/V2 slices emitted)
            v0u = V[:, 512 + 64 * u:512 + 64 * (u + 1)]
            v1u = V[:, 256:512].rearrange(
                "c (sec b n) -> c sec b n", sec=2, b=BL
            )[:, :, u, :]
            v2u = V[:, 0:256].rearrange(
                "c (sec b n) -> c sec b n", sec=2, b=BL
            )[:, :, u, :]
            sl = slice(u * 64, (u + 1) * 64)
            colsum(ps_d01[:, sl], v0u, v1u, 64, f"d01{u}")
            colsum(ps_d02[:, sl], v0u, v2u, 64, f"d02{u}")

        gatherL(2, T2, 0, 0, 0, "2")
        stage_B()
        gatherL(1, T1, 32, 256, 512, "1")
        colsum(ps_d12[:], V[:, 256:512], V[:, 0:256], 256, "d12")
        for u in range(BL):
            gather0(u)
            dots0(u)

        # ---- epilogue on partition 0 (reads PSUM directly) ----
        ssc = pool.tile([1, 384], F32, name="ssc")
        dc = pool.tile([1, 384], F32, name="dc")

        def secsum(dst, src, l0_layout):
            # single-input reduce over the chunk axis (PSUM-legal)
            if l0_layout:  # src [1, 256] cols (u, sec, n)
                v = src.rearrange("o (u sec n) -> o u n sec", u=BL, sec=2)
                nc.vector.tensor_reduce(
                    out=dst.rearrange("o (u n) -> o u n", u=BL),
                    in_=v, axis=mybir.AxisListType.X, op=AL.add,
                )
            else:  # src [1, 256] cols (sec, b, n)
                nc.vector.tensor_reduce(
                    out=dst,
                    in_=src.rearrange("o (sec n) -> o n sec", sec=2),
                    axis=mybir.AxisListType.X, op=AL.add,
                )

        secsum(SEG(ssc, 0), ps_ss0[:], True)
        secsum(SEG(ssc, 1), ps_ss[:, 256:512], False)
        secsum(SEG(ssc, 2), ps_ss[:, 0:256], False)
        secsum(SEG(dc, 0), ps_d01[:], True)
        secsum(SEG(dc, 1), ps_d02[:], True)
        secsum(SEG(dc, 2), ps_d12[:], False)

        # rn = 1 / max(sqrt(ssc), EPS) == 1 / sqrt(max(ssc, EPS^2))
        nc.vector.tensor_scalar_max(out=ssc[:], in0=ssc[:], scalar1=EPS * EPS)
        nrm = pool.tile([1, 384], F32, name="nrm")
        nc.scalar.sqrt(out=nrm[:], in_=ssc[:])
        rn = pool.tile([1, 384], F32, name="rn")
        nc.vector.reciprocal(out=rn[:], in_=nrm[:])

        rp = pool.tile([1, 384], F32, name="rp")
        for seg, (i, j) in enumerate(PAIRS):
            nc.vector.tensor_tensor(
                out=SEG(rp, seg), in0=SEG(rn, i), in1=SEG(rn, j), op=AL.mult
            )
        nc.vector.tensor_tensor(out=dc[:], in0=dc[:], in1=rp[:], op=AL.mult)
        res = pool.tile([1, 1], F32)
        nc.vector.tensor_reduce(
            out=res[:], in_=dc[:], axis=mybir.AxisListType.X, op=AL.add
        )
        nc.sync.dma_start(out=out.ap(), in_=res[:])

    nc.compile()
    return nc


def _get_program():
    if "nc" not in _CACHE:
        _CACHE["nc"] = _build_program()
    return _CACHE["nc"]


def _run_device(feat0, feat1, feat2, boxes, **run_kwargs):
    from concourse.bass_utils import run_bass_kernel_spmd

    nc = _get_program()

    feats = [
        np.ascontiguousarray(np.asarray(f, dtype=np.float32))
        for f in (feat0, feat1, feat2)
    ]
    boxes = np.ascontiguousarray(np.asarray(boxes, dtype=np.float32))

    in_maps = []
    for k in range(N_CORES):
        sl = slice(k * BL, (k + 1) * BL)
        in_maps.append(
            {
                "feat0": feats[0][sl],
                "feat1": feats[1][sl],
                "feat2": feats[2][sl],
                "boxes": boxes[sl],
            }
        )

    return run_bass_kernel_spmd(
        nc, in_maps, core_ids=list(range(N_CORES)), **run_kwargs
    )


def kernel(feat0, feat1, feat2, boxes):
    r = _run_device(feat0, feat1, feat2, boxes)
    total = np.float64(0.0)
    for m in r.results:
        total += np.float64(m["out"].reshape(-1)[0])

    count = B * N * len(PAIRS)
    avg = np.float32(total) / np.float32(count)
    loss = np.float32(1.0) - avg
    loss = np.nan_to_num(loss, nan=0.0, posinf=1.0, neginf=0.0)
    return np.array(np.clip(loss, 0.0, 2.0), dtype=np.float32)
